# revision 1
# baseline (speedup 1.0000x reference)
"""DMGCN message-passing GNN on 8 Trainium2 NeuronCores (Bass/Tile).

Sharding: edges sorted by dst; core c owns nodes [c*12500,(c+1)*12500) and the
edges targeting them. Per layer: node-MLP on own node shard -> AllGather hn
table -> edge phase (indirect-DMA gather hn[src], edge MLP, message matmul,
one-hot scatter matmuls accumulating in PSUM) -> h update. Readout on device,
graph segment-sum on host (unshard step).
"""
import os
import sys

for _p in ("/opt/trn_rl_repo", "/root/.axon_site/_ro/trn_rl_repo"):
    if os.path.isdir(_p) and _p not in sys.path:
        sys.path.insert(0, _p)

import numpy as np
import concourse.bass as bass
import concourse.mybir as mybir
import concourse.tile as tile
from concourse.bass_utils import run_bass_kernel_spmd
from concourse.masks import make_identity

# problem constants (hardcoded per spec)
N, E, G = 100000, 400000, 2000
D = 128
NC = 300           # RBF centers
CUT_LO, CUT_HI = 0.0, 30.0
N_CONV = 3
NCORES = 8
P = 128
N_SH = N // NCORES            # 12500 nodes per core
NT = (N_SH + P - 1) // P      # 98 node tiles per core
N_PAD = NT * P                # 12544 padded hn-table rows per core
DE = 428

F32 = mybir.dt.float32
I32 = mybir.dt.int32
AF = mybir.ActivationFunctionType
ALU = mybir.AluOpType

PAD_OFF = 200.0               # dst_off sentinel for padded edges
ABLATE = frozenset()          # timing ablations: nocc, nogather, nozchain, noconsume, nostoree
F16 = mybir.dt.float16        # matmul dtype for the edge-MLP / message chain
NP16 = "float16"              # host-side dtype string

# K-chunking of the 428-dim edge feature axis: emb 0:128 | rbf 128:428
KCH = [(0, 128), (128, 256), (256, 384), (384, 428)]     # z1 M-chunks / z2 K-chunks
VCH = [(0, 128), (128, 256), (256, 300)]                 # rbf center chunks


def split_waits(nc):
    """Walrus allows only 1 sync wait per instruction; hoist extras onto
    preceding NoOps on the same engine."""
    n_fix = 0
    for f in nc.m.functions:
        for blk in f.blocks:
            out = []
            for inst in blk.instructions:
                si = inst.sync_info
                if si and len(si.on_wait) > 1 and not isinstance(inst, mybir.InstNoOp):
                    waits = list(si.on_wait)
                    for w in waits[:-1]:
                        nop = mybir.InstNoOp(name=f"{inst.name}-ws{n_fix}", ins=[], outs=[])
                        nop.engine = inst.engine
                        nop.sync_info = mybir.SyncInfo(on_wait=[w], on_update=[])
                        out.append(nop)
                        n_fix += 1
                    si.on_wait = [waits[-1]]
                out.append(inst)
            blk.instructions[:] = out
    return n_fix


def host_prep(inputs):
    """Sort/shard edges, build per-core arrays and weight layouts."""
    Z = np.asarray(inputs["Z"]).astype(np.int32)
    edge_type = np.asarray(inputs["edge_type"]).astype(np.int32)
    dist = np.asarray(inputs["dist"]).astype(np.float32)
    src = np.asarray(inputs["src"]).astype(np.int64)
    dst = np.asarray(inputs["dst"]).astype(np.int64)
    graph_ids = np.asarray(inputs["graph_ids"]).astype(np.int64)

    order = np.argsort(dst, kind="stable")
    dsts = dst[order]
    srcs = src[order]
    dists = dist[order]
    etypes = edge_type[order]

    core_lo = np.searchsorted(dsts, np.arange(NCORES) * N_SH, side="left")
    core_hi = np.append(core_lo[1:], E)

    tile_cnt = np.zeros((NCORES, NT), dtype=np.int64)
    for c in range(NCORES):
        dl = dsts[core_lo[c]:core_hi[c]] - c * N_SH
        tile_cnt[c] = np.bincount(dl // P, minlength=NT)
    tmax = max(1, int(np.max((tile_cnt + P - 1) // P)))

    n_sub = NT * tmax                      # 128-edge sub-tiles per core
    n512 = (n_sub + 3) // 4                # 512-edge z-tiles per core
    n_sub_pad = n512 * 4
    e_slots = n_sub_pad * P

    # global padded row index into the allgathered hn table
    src_row = ((srcs // N_SH) * N_PAD + (srcs % N_SH)).astype(np.int32)

    def to_pf(arr):
        # [e_slots] -> [128, n_sub_pad]; element (p, s) = arr[s*128 + p]
        return np.ascontiguousarray(arr.reshape(n_sub_pad, P).T)

    core_in = []
    for c in range(NCORES):
        lo, hi = core_lo[c], core_hi[c]
        dl = (dsts[lo:hi] - c * N_SH).astype(np.int64)
        sr = np.zeros(e_slots, dtype=np.int32)
        doff = np.full(e_slots, PAD_OFF, dtype=np.float32)
        dd = np.zeros(e_slots, dtype=np.float32)
        et = np.zeros(e_slots, dtype=np.int32)
        start = 0
        for t in range(NT):
            cnt = int(tile_cnt[c, t])
            base = t * tmax * P
            sl = slice(start, start + cnt)
            sr[base:base + cnt] = src_row[lo:hi][sl]
            doff[base:base + cnt] = (dl[sl] - t * P).astype(np.float32)
            dd[base:base + cnt] = dists[lo:hi][sl]
            et[base:base + cnt] = etypes[lo:hi][sl]
            start += cnt
        assert start == hi - lo
        x3 = np.stack([dd, dd * dd, np.ones_like(dd)], 0).astype(np.float32)
        Zc = np.zeros(N_PAD, dtype=np.int32)
        Zc[:N_SH] = Z[c * N_SH:(c + 1) * N_SH]
        core_in.append(dict(
            src_row=to_pf(sr), dst_off=to_pf(doff), x3=x3, etype=to_pf(et),
            z_idx=np.ascontiguousarray(Zc.reshape(NT, P).T),
        ))

    w = {}
    centers = np.linspace(CUT_LO, CUT_HI, NC, dtype=np.float32)
    gap = np.float32(centers[1] - centers[0])
    w["A"] = np.stack([2.0 * centers / gap,
                       -np.ones(NC, np.float32) / gap,
                       -(centers ** 2) / gap], 0).astype(np.float32)   # [3, NC]
    w["node_emb"] = np.asarray(inputs["node_emb"]).astype(np.float32)
    w["edge_emb"] = np.asarray(inputs["edge_emb"]).astype(np.float32)
    for i in range(N_CONV):
        w[f"wn1t_{i}"] = np.ascontiguousarray(np.asarray(inputs["Wn1"][i]).T.astype(np.float32))
        w[f"wn2t_{i}"] = np.ascontiguousarray(np.asarray(inputs["Wn2"][i]).T.astype(np.float32))
        w[f"we1t_{i}"] = np.ascontiguousarray(np.asarray(inputs["We1"][i]).T.astype(NP16))
        w[f"we2t_{i}"] = np.ascontiguousarray(np.asarray(inputs["We2"][i]).T.astype(NP16))
        w[f"wct_{i}"] = np.ascontiguousarray(np.asarray(inputs["Wc"][i]).T.astype(NP16))
        w[f"bn1_{i}"] = np.asarray(inputs["bn1"][i]).reshape(D, 1).astype(np.float32)
        w[f"bn2_{i}"] = np.asarray(inputs["bn2"][i]).reshape(D, 1).astype(np.float32)
        w[f"be1_{i}"] = np.asarray(inputs["be1"][i]).reshape(DE, 1).astype(np.float32)
        w[f"be2_{i}"] = np.asarray(inputs["be2"][i]).reshape(D, 1).astype(np.float32)
        w[f"bc_{i}"] = np.ascontiguousarray(
            np.tile(np.asarray(inputs["bc"][i]).reshape(1, D), (1, 4))).astype(NP16)
    w["wr1t"] = np.ascontiguousarray(np.asarray(inputs["Wr1"]).T.astype(np.float32))
    w["wr2t"] = np.ascontiguousarray(np.asarray(inputs["Wr2"]).T.astype(np.float32))
    w["br1"] = np.asarray(inputs["br1"]).reshape(D, 1).astype(np.float32)
    w["br2"] = np.full((D, 1), np.asarray(inputs["br2"]).reshape(()),
                       dtype=np.float32)

    meta = dict(tmax=tmax, n_sub=n_sub, n512=n512, n_sub_pad=n_sub_pad,
                e_slots=e_slots)
    return core_in, w, meta, graph_ids


def build_nc(meta, reps=1):
    tmax, n512, n_sub = meta["tmax"], meta["n512"], meta["n_sub"]
    n_sub_pad, e_slots = meta["n_sub_pad"], meta["e_slots"]

    nc = bass.Bass(num_devices=NCORES)

    t_in = {}

    def inp(name, shp, dt=F32):
        t_in[name] = nc.dram_tensor(name, shp, dt, kind="ExternalInput")
        return t_in[name]

    src_row = inp("src_row", [P, n_sub_pad], I32)
    dst_off = inp("dst_off", [P, n_sub_pad], F32)
    x3 = inp("x3", [3, e_slots], F32)
    etype = inp("etype", [P, n_sub_pad], I32)
    z_idx = inp("z_idx", [P, NT], I32)
    A_t = inp("A", [3, NC], F32)
    node_emb = inp("node_emb", [20, D], F32)
    edge_emb = inp("edge_emb", [400, D], F32)
    for i in range(N_CONV):
        for nm, shp in (("wn1t", [D, D]), ("wn2t", [D, D]), ("bn1", [D, 1]),
                        ("bn2", [D, 1]), ("be1", [DE, 1]), ("be2", [D, 1])):
            inp(f"{nm}_{i}", shp)
        for nm, shp in (("we1t", [DE, DE]), ("we2t", [DE, D]), ("wct", [D, D]),
                        ("bc", [1, 4 * D])):
            inp(f"{nm}_{i}", shp, F16)
    inp("wr1t", [D, D]); inp("wr2t", [D, 1]); inp("br1", [D, 1]); inp("br2", [D, 1])
    r_out = nc.dram_tensor("r_out", [N_PAD, 1], F32, kind="ExternalOutput")

    e_fm = nc.dram_tensor("e_fm", [P, e_slots], F16, kind="Internal")
    ee_dram = [nc.dram_tensor(f"ee_{i}", [P, e_slots], F16, kind="Internal")
               for i in range(N_CONV)]
    cc_in = [nc.dram_tensor(f"cc_in_{i}", [N_PAD, D], F16, kind="Internal")
             for i in range(N_CONV)]
    cc_out = [nc.dram_tensor(f"cc_out_{i}", [NCORES * N_PAD, D], F16,
                             kind="Internal", addr_space="Shared")
              for i in range(N_CONV)]

    with tile.TileContext(nc) as tc:
        with (
            tc.tile_pool(name="const", bufs=1) as cp,
            tc.tile_pool(name="sb", bufs=4) as sb,
            tc.tile_pool(name="gat", bufs=6) as gp,
            tc.tile_pool(name="zr", bufs=2) as zp,
            tc.tile_pool(name="pv", bufs=1, space="PSUM") as pv,
            tc.tile_pool(name="pz1", bufs=2, space="PSUM") as pz1,
            tc.tile_pool(name="pz2", bufs=1, space="PSUM") as pz2,
            tc.tile_pool(name="ptp", bufs=1, space="PSUM") as ptp,
            tc.tile_pool(name="pm", bufs=1, space="PSUM") as pm,
            tc.tile_pool(name="pd", bufs=1, space="PSUM") as pd,
        ):
            # ---------------- constants in SBUF ----------------
            ident = cp.tile([P, P], F32)
            make_identity(nc, ident[:])
            ident16 = cp.tile([P, P], F16)
            nc.vector.tensor_copy(out=ident16[:], in_=ident[:])
            iota_i = cp.tile([P, P], I32)
            nc.gpsimd.iota(iota_i[:], pattern=[[1, P]], base=0, channel_multiplier=0)
            iota_f = cp.tile([P, P], F32)
            nc.vector.tensor_copy(out=iota_f[:], in_=iota_i[:])
            ones_row = cp.tile([1, P], F16)
            nc.vector.memset(ones_row[:], 1.0)

            def load_const(name, shp):
                tl = cp.tile(shp, F32, tag=name)
                nc.sync.dma_start(out=tl[:], in_=t_in[name][:, :])
                return tl

            A_sb = load_const("A", [3, NC])
            wr1t_sb = load_const("wr1t", [D, D])
            wr2t_sb = load_const("wr2t", [D, 1])
            br1_sb = load_const("br1", [D, 1])
            br2_sb = load_const("br2", [D, 1])
            W = {}
            for i in range(N_CONV):
                for nm in ("wn1t", "wn2t", "bn1", "bn2", "be2"):
                    shp = {"wn1t": [D, D], "wn2t": [D, D],
                           "bn1": [D, 1], "bn2": [D, 1], "be2": [D, 1]}[nm]
                    W[f"{nm}_{i}"] = load_const(f"{nm}_{i}", shp)
                for nm, shp in (("wct", [D, D]), ("bc", [1, 4 * D])):
                    tl = cp.tile(shp, F16, tag=f"{nm}_{i}", name=f"{nm}_{i}")
                    nc.sync.dma_start(out=tl[:], in_=t_in[f"{nm}_{i}"][:, :])
                    W[f"{nm}_{i}"] = tl
                # we1t [428,428] -> K-chunk tiles [<=128, 428]
                for k, (k0, k1) in enumerate(KCH):
                    tl = cp.tile([k1 - k0, DE], F16, tag=f"we1t_{i}_{k}")
                    nc.sync.dma_start(out=tl[:], in_=t_in[f"we1t_{i}"][k0:k1, :])
                    W[f"we1t_{i}_{k}"] = tl
                    tl2 = cp.tile([k1 - k0, D], F16, tag=f"we2t_{i}_{k}")
                    nc.sync.dma_start(out=tl2[:], in_=t_in[f"we2t_{i}"][k0:k1, :])
                    W[f"we2t_{i}_{k}"] = tl2
                be1 = cp.tile([P, 4], F32, tag=f"be1_{i}")  # 4 chunk-columns
                for k, (k0, k1) in enumerate(KCH):
                    nc.sync.dma_start(out=be1[:k1 - k0, k:k + 1],
                                      in_=t_in[f"be1_{i}"][k0:k1, :])
                W[f"be1_{i}"] = be1

            # persistent h in SBUF, feature-major [128, N_PAD]
            h_fm = cp.tile([P, N_PAD], F32, tag="h_fm")

            # dst_off + src_row resident in SBUF (used each layer)
            doff_sb = cp.tile([P, n_sub_pad], F32, tag="doff_sb")
            nc.sync.dma_start(out=doff_sb[:], in_=dst_off[:, :])
            sri = cp.tile([P, n_sub_pad], I32, tag="sri_sb")
            nc.sync.dma_start(out=sri[:], in_=src_row[:, :])
            zi = cp.tile([P, NT], I32, tag="z_idx_sb")
            nc.sync.dma_start(out=zi[:], in_=z_idx[:, :])
            eti = cp.tile([P, n_sub_pad], I32, tag="etype_sb")
            nc.sync.dma_start(out=eti[:], in_=etype[:, :])

            # ---------------- helpers ----------------
            def gather_transpose_to(dst_tile, dst_col, table, idx_col):
                """Gather 128 rows of `table` by idx_col [128,1] -> transpose ->
                write into dst_tile[:, dst_col:dst_col+128] (feature-major)."""
                g = gp.tile([P, D], F32, tag="gath")
                nc.gpsimd.indirect_dma_start(
                    out=g[:], out_offset=None, in_=table[:, :],
                    in_offset=bass.IndirectOffsetOnAxis(ap=idx_col, axis=0))
                pt = ptp.tile([P, P], F32, space="PSUM", tag="tp")
                nc.tensor.transpose(out=pt[:], in_=g[:], identity=ident[:])
                nc.scalar.copy(out=dst_tile[:, dst_col:dst_col + P], in_=pt[:])

            def gather_transpose16(pt_wide, dst_col, table, idx_col):
                """fp16 gather + PE-transpose into a slice of a wide fp16 PSUM tile."""
                g = gp.tile([P, D], F16, tag="gath16")
                if "nogather" in ABLATE:
                    nc.gpsimd.dma_start(out=g[:], in_=table[0:P, :])
                else:
                    nc.gpsimd.indirect_dma_start(
                        out=g[:], out_offset=None, in_=table[:, :],
                        in_offset=bass.IndirectOffsetOnAxis(ap=idx_col, axis=0))
                nc.tensor.transpose(out=pt_wide[:, dst_col:dst_col + P], in_=g[:],
                                    identity=ident16[:])

            for rep_ in range(reps):
                # ---------------- prologue: h0 init ----------------
                for t in range(NT):
                    gather_transpose_to(h_fm, t * P, node_emb, zi[:, t:t + 1])

                # ---------------- prologue: e_fm build ----------------
                for j in range(n512):
                    ef = sb.tile([P, 4 * P], F16, tag="ef_build")
                    for a in range(4):
                        s = j * 4 + a
                        gather_transpose_to(ef, a * P, edge_emb, eti[:, s:s + 1])
                    nc.sync.dma_start(out=e_fm[:, j * 512:(j + 1) * 512], in_=ef[:])

                # ---------------- layers ----------------
                for i in range(N_CONV):
                    # --- node MLP: hn = relu(Wn1@h + bn1); Wn2@ + bn2 ---
                    for j0 in range(0, N_PAD, 512):
                        wdt = min(512, N_PAD - j0)
                        ps1 = pz1.tile([P, 512], F32, space="PSUM", tag="pz1")
                        nc.tensor.matmul(out=ps1[:, :wdt], lhsT=W[f"wn1t_{i}"][:],
                                         rhs=h_fm[:, j0:j0 + wdt], start=True, stop=True)
                        zb = sb.tile([P, 512], F32, tag="nmlp_z")
                        nc.scalar.activation(out=zb[:, :wdt], in_=ps1[:, :wdt],
                                             func=AF.Relu, bias=W[f"bn1_{i}"][:, :1])
                        ps2 = pz2.tile([P, 512], F32, space="PSUM", tag="pz2")
                        nc.tensor.matmul(out=ps2[:, :wdt], lhsT=W[f"wn2t_{i}"][:],
                                         rhs=zb[:, :wdt], start=True, stop=True)
                        hnb = sb.tile([P, 512], F32, tag="nmlp_hn")
                        nc.scalar.activation(out=hnb[:, :wdt], in_=ps2[:, :wdt],
                                             func=AF.Identity, bias=W[f"bn2_{i}"][:, :1])
                        # transpose to node-major and ship to cc_in
                        for a in range(wdt // P):
                            pt = ptp.tile([P, P], F32, space="PSUM", tag="tp")
                            nc.tensor.transpose(out=pt[:], in_=hnb[:, a * P:(a + 1) * P],
                                                identity=ident[:])
                            hnm = sb.tile([P, P], F16, tag="hn_nm")
                            nc.vector.tensor_copy(out=hnm[:], in_=pt[:])
                            nc.sync.dma_start(
                                out=cc_in[i][j0 + a * P:j0 + (a + 1) * P, :], in_=hnm[:])

                    if "nocc" not in ABLATE:
                        nc.gpsimd.collective_compute(
                            "AllGather", ALU.bypass,
                            replica_groups=[list(range(NCORES))],
                            ins=[cc_in[i][:, :]], outs=[cc_out[i][:, :]])

                    # --- ee z-chain (no dependence on h / collective) ---
                    x3q = None
                    for j in range(n512 if "nozchain" not in ABLATE else 0):
                        js = slice(j * 512, (j + 1) * 512)
                        if j % 4 == 0:
                            x3q = sb.tile([3, 2048], F32, tag="x3q")
                            qw = min(2048, e_slots - j * 512)
                            nc.sync.dma_start(out=x3q[:, :qw],
                                              in_=x3[:, j * 512:j * 512 + qw])
                        x3t = x3q[:, (j % 4) * 512:(j % 4 + 1) * 512]
                        eft = sb.tile([P, 512], F16, tag="eft")
                        nc.sync.dma_start(out=eft[:], in_=e_fm[:, js])
                        # V chunks = exp(A.T @ x3)
                        vch = []
                        for k, (c0, c1) in enumerate(VCH):
                            pvt = pv.tile([P, 512], F32, space="PSUM", tag="pv")
                            nc.tensor.matmul(out=pvt[:c1 - c0, :], lhsT=A_sb[:, c0:c1],
                                             rhs=x3t, start=True, stop=True)
                            vt = sb.tile([P, 512], F16, tag=f"vch{k}")
                            nc.scalar.activation(out=vt[:c1 - c0, :], in_=pvt[:c1 - c0, :],
                                                 func=AF.Exp)
                            vch.append(vt)
                        # z1 M-chunks, K = emb(128) + V(300)
                        z1r = []
                        for mi, (m0, m1) in enumerate(KCH):
                            pz = pz1.tile([P, 512], F32, space="PSUM", tag="pz1")
                            nc.tensor.matmul(out=pz[:m1 - m0, :],
                                             lhsT=W[f"we1t_{i}_0"][:, m0:m1],
                                             rhs=eft[:], start=True, stop=False)
                            for k, (c0, c1) in enumerate(VCH):
                                nc.tensor.matmul(
                                    out=pz[:m1 - m0, :],
                                    lhsT=W[f"we1t_{i}_{k + 1}"][:c1 - c0, m0:m1],
                                    rhs=vch[k][:c1 - c0, :],
                                    start=False, stop=(k == len(VCH) - 1))
                            zr_t = zp.tile([P, 512], F16, tag=f"z1r{mi}")
                            eng = nc.scalar if mi < 2 else nc.vector
                            if mi < 2:
                                nc.scalar.activation(out=zr_t[:m1 - m0, :], in_=pz[:m1 - m0, :],
                                                     func=AF.Relu,
                                                     bias=W[f"be1_{i}"][:m1 - m0, mi:mi + 1])
                            else:
                                nc.vector.tensor_scalar(
                                    out=zr_t[:m1 - m0, :], in0=pz[:m1 - m0, :],
                                    scalar1=W[f"be1_{i}"][:m1 - m0, mi:mi + 1],
                                    scalar2=0.0, op0=ALU.add, op1=ALU.max)
                            z1r.append(zr_t)
                        # z2 = We2 @ z1r + be2 -> ee
                        pe = pz2.tile([P, 512], F32, space="PSUM", tag="pz2")
                        for k, (k0, k1) in enumerate(KCH):
                            nc.tensor.matmul(out=pe[:], lhsT=W[f"we2t_{i}_{k}"][:],
                                             rhs=z1r[k][:k1 - k0, :],
                                             start=(k == 0), stop=(k == len(KCH) - 1))
                        eet = sb.tile([P, 512], F16, tag="eet")
                        nc.scalar.activation(out=eet[:], in_=pe[:], func=AF.Identity,
                                             bias=W[f"be2_{i}"][:, :1])
                        if "nostoree" not in ABLATE:
                            nc.sync.dma_start(out=ee_dram[i][:, js], in_=eet[:])

                    # --- consume: gather hn, product, message, scatter ---
                    cur_pd = [None]
                    for j in range(n512 if "noconsume" not in ABLATE else 0):
                        js = slice(j * 512, (j + 1) * 512)
                        eet = sb.tile([P, 512], F16, tag="eet_c")
                        nc.sync.dma_start(out=eet[:], in_=ee_dram[i][:, js])
                        n_active = min(4, n_sub - j * 4)
                        ptw = ptp.tile([P, 512], F16, space="PSUM", tag="tpw")
                        for a in range(n_active):
                            s = j * 4 + a
                            gather_transpose16(ptw, a * P, cc_out[i], sri[:, s:s + 1])
                        hnf = sb.tile([P, 512], F16, tag="hnf")
                        nc.vector.tensor_copy(out=hnf[:, :n_active * P],
                                              in_=ptw[:, :n_active * P])
                        prod = sb.tile([P, 512], F16, tag="prod")
                        nc.vector.tensor_mul(out=prod[:, :n_active * P], in0=eet[:, :n_active * P],
                                             in1=hnf[:, :n_active * P])
                        pmw = pm.tile([P, 512], F32, space="PSUM", tag="pm")
                        nc.tensor.matmul(out=pmw[:, :n_active * P], lhsT=ones_row[:],
                                         rhs=W[f"bc_{i}"][:, :n_active * P],
                                         start=True, stop=False)
                        for a in range(n_active):
                            nc.tensor.matmul(out=pmw[:, a * P:(a + 1) * P],
                                             lhsT=prod[:, a * P:(a + 1) * P],
                                             rhs=W[f"wct_{i}"][:], start=False,
                                             stop=True, skip_group_check=True)
                        msb = sb.tile([P, 512], F16, tag="msb")
                        nc.scalar.activation(out=msb[:, :n_active * P],
                                             in_=pmw[:, :n_active * P], func=AF.Tanh)
                        for a in range(n_active):
                            s = j * 4 + a
                            t_node = s // tmax
                            pos = s % tmax
                            if pos == 0:
                                cur_pd[0] = pd.tile([P, P], F32, space="PSUM", tag="pd", name="pdt")
                            S = sb.tile([P, P], F16, tag="S")
                            nc.vector.tensor_tensor(
                                out=S[:], in0=doff_sb[:, s:s + 1].to_broadcast([P, P]),
                                in1=iota_f[:], op=ALU.is_equal)
                            pdt = cur_pd[0]
                            nc.tensor.matmul(out=pdt[:], lhsT=S[:],
                                             rhs=msb[:, a * P:(a + 1) * P],
                                             start=(pos == 0), stop=(pos == tmax - 1))
                            if pos == tmax - 1:
                                dsb = sb.tile([P, P], F32, tag="dsb")
                                nc.vector.tensor_copy(out=dsb[:], in_=pdt[:])
                                pt = ptp.tile([P, P], F32, space="PSUM", tag="tp")
                                nc.tensor.transpose(out=pt[:], in_=dsb[:], identity=ident[:])
                                nc.vector.tensor_add(
                                    out=h_fm[:, t_node * P:(t_node + 1) * P],
                                    in0=h_fm[:, t_node * P:(t_node + 1) * P], in1=pt[:])

                # ---------------- readout ----------------
                for j0 in range(0, N_PAD, 512):
                    wdt = min(512, N_PAD - j0)
                    ps1 = pz1.tile([P, 512], F32, space="PSUM", tag="pz1")
                    nc.tensor.matmul(out=ps1[:, :wdt], lhsT=wr1t_sb[:],
                                     rhs=h_fm[:, j0:j0 + wdt], start=True, stop=True)
                    qb = sb.tile([P, 512], F32, tag="qb")
                    nc.scalar.activation(out=qb[:, :wdt], in_=ps1[:, :wdt],
                                         func=AF.Relu, bias=br1_sb[:, :1])
                    for a in range(wdt // P):
                        prt = pm.tile([P, P], F32, space="PSUM", tag="pm")
                        nc.tensor.matmul(out=prt[:, :1], lhsT=qb[:, a * P:(a + 1) * P],
                                         rhs=wr2t_sb[:], start=True, stop=True)
                        rsb = sb.tile([P, 1], F32, tag="rsb")
                        nc.scalar.activation(out=rsb[:], in_=prt[:, :1], func=AF.Identity,
                                             bias=br2_sb[:, :1])
                        nc.sync.dma_start(out=r_out[j0 + a * P:j0 + (a + 1) * P, :],
                                          in_=rsb[:])
    return nc


_CACHE = {}


def _get_runner(meta, reps=1):
    key = (tuple(sorted(meta.items())), reps, ABLATE)
    if key not in _CACHE:
        nc = build_nc(meta, reps=reps)
        nc.finalize()
        split_waits(nc)
        _CACHE[key] = nc
    return _CACHE[key]


def kernel(**inputs):
    core_in, w, meta, graph_ids = host_prep(inputs)
    nc = _get_runner(meta)
    in_maps = []
    for c in range(NCORES):
        m = dict(core_in[c])
        m.update(w)
        in_maps.append(m)
    res = run_bass_kernel_spmd(nc, in_maps, core_ids=list(range(NCORES)))
    r = np.concatenate([res.results[c]["r_out"][:N_SH, 0] for c in range(NCORES)])
    out = np.bincount(graph_ids, weights=r.astype(np.float64), minlength=G)[:G]
    return out.astype(np.float32)



# revision 2
# speedup vs baseline: 1.4544x; 1.4544x over previous
"""DMGCN v2: packed equalized geometry, fused z+consume, dma_gather prologue.

Sharding: edges sorted by dst; core c owns nodes [c*12500,(c+1)*12500). Edge
slots follow a uniform per-tile profile q_t = max_c cnt_{c,t} so the
subtile->window scatter schedule is compile-time and shared by all cores.
Per layer: node-MLP (fp16) on own shard -> AllGather hn -> fused per-512-tile
[edge-MLP z-chain -> gather hn[src] -> product -> message matmul -> one-hot
scatter into PSUM windows] -> h update. Readout on device; graph segment-sum
on host.
"""
import os
import sys

for _p in ("/opt/trn_rl_repo", "/root/.axon_site/_ro/trn_rl_repo"):
    if os.path.isdir(_p) and _p not in sys.path:
        sys.path.insert(0, _p)

import numpy as np
import concourse.bass as bass
import concourse.mybir as mybir
import concourse.tile as tile
from concourse.bass_utils import run_bass_kernel_spmd
from concourse.masks import make_identity

N, E, G = 100000, 400000, 2000
D = 128
NC = 300
CUT_LO, CUT_HI = 0.0, 30.0
N_CONV = 3
NCORES = 8
P = 128
N_SH = N // NCORES            # 12500
NT = (N_SH + P - 1) // P      # 98
N_PAD = NT * P                # 12544
DE = 428

F32 = mybir.dt.float32
F16 = mybir.dt.float16
I32 = mybir.dt.int32
I16 = mybir.dt.int16
AF = mybir.ActivationFunctionType
ALU = mybir.AluOpType

PAD_OFF = 999.0
ABLATE = frozenset({"nodg"})
NP16 = "float16"
F8 = mybir.dt.float8e4

KCH = [(0, 128), (128, 256), (256, 384), (384, 428)]
VCH = [(0, 128), (128, 256), (256, 300)]

EG_GRP = 4            # 512-tiles per dma_gather launch (2048 edges)
SEG = 8192            # edge slots per hn-gather epoch (16 tiles)
H0_GRP = 2048         # h0 gather batch


def split_waits(nc):
    """Walrus allows only 1 sync wait per instruction; hoist extras onto
    preceding NoOps on the same engine."""
    n_fix = 0
    for f in nc.m.functions:
        for blk in f.blocks:
            out = []
            for inst in blk.instructions:
                si = inst.sync_info
                if si and len(si.on_wait) > 1 and not isinstance(inst, mybir.InstNoOp):
                    waits = list(si.on_wait)
                    for w in waits[:-1]:
                        nop = mybir.InstNoOp(name=f"{inst.name}-ws{n_fix}", ins=[], outs=[])
                        nop.engine = inst.engine
                        nop.sync_info = mybir.SyncInfo(on_wait=[w], on_update=[])
                        out.append(nop)
                        n_fix += 1
                    si.on_wait = [waits[-1]]
                out.append(inst)
            blk.instructions[:] = out
    return n_fix


def finalize_v2(nc):
    """finalize + ucode library loads + extended-inst codegen + wait splits."""
    nc.finalize()
    import bass_rust
    from concourse.library_config import all_libraries, standard
    from concourse.library_overlay import lower_extended_insts
    mask = {}
    for lib in all_libraries:
        for it in lib.instructions:
            mask[it] = mask.get(it, 0) | (1 << lib.index)
    bass_rust.insert_library_loads(nc, mask, len(all_libraries), standard.index)
    lower_extended_insts(nc)
    split_waits(nc)
    return nc


def wrap16(idx):
    """token i -> [i % 16, i // 16], replicated across the 8 Q7 cores."""
    n = len(idx)
    assert n % 16 == 0
    w = np.asarray(idx, np.int16).reshape(n // 16, 16).T
    return np.ascontiguousarray(np.tile(w, (8, 1)))


def geometry(q):
    """Compile-time maps from the uniform slot profile q[98]."""
    q = np.asarray(q, np.int64)
    Q = np.concatenate([[0], np.cumsum(q)])
    slots = int(Q[NT])
    e_slots = ((slots + 511) // 512) * 512
    n_sub = e_slots // 128
    n512 = e_slots // 512
    tiles_of, w1 = [], []
    for s in range(n_sub):
        lo, hi = 128 * s, 128 * (s + 1)
        ts = [t for t in range(NT) if Q[t] < hi and Q[t + 1] > lo]
        assert len(ts) <= 2, (s, ts)
        tiles_of.append(ts)
        w1.append(ts[0] if ts else -1)
    s_first = {t: int(Q[t] // 128) for t in range(NT)}
    s_last = {t: int((Q[t + 1] - 1) // 128) for t in range(NT)}
    return Q, e_slots, n_sub, n512, tiles_of, w1, s_first, s_last


def host_prep(inputs):
    Z = np.asarray(inputs["Z"]).astype(np.int64)
    edge_type = np.asarray(inputs["edge_type"]).astype(np.int64)
    dist = np.asarray(inputs["dist"]).astype(np.float32)
    src = np.asarray(inputs["src"]).astype(np.int64)
    dst = np.asarray(inputs["dst"]).astype(np.int64)
    graph_ids = np.asarray(inputs["graph_ids"]).astype(np.int64)

    order = np.argsort(dst, kind="stable")
    dsts = dst[order]
    srcs = src[order]
    dists = dist[order]
    etypes = edge_type[order]

    core_lo = np.searchsorted(dsts, np.arange(NCORES) * N_SH, side="left")
    core_hi = np.append(core_lo[1:], E)

    cnt = np.zeros((NCORES, NT), dtype=np.int64)
    for c in range(NCORES):
        dl = dsts[core_lo[c]:core_hi[c]] - c * N_SH
        cnt[c] = np.bincount(dl // P, minlength=NT)
    q = cnt.max(axis=0)
    Q, e_slots, n_sub, n512, tiles_of, w1, s_first, s_last = geometry(q)

    src_row = ((srcs // N_SH) * N_PAD + (srcs % N_SH)).astype(np.int32)

    def to_pf(arr, width):
        return np.ascontiguousarray(arr.reshape(width, P).T)

    core_in = []
    for c in range(NCORES):
        lo = core_lo[c]
        sr = np.zeros(e_slots, dtype=np.int32)
        doff = np.full(e_slots, PAD_OFF, dtype=np.float32)
        dd = np.zeros(e_slots, dtype=np.float32)
        et = np.zeros(e_slots, dtype=np.int64)
        start = 0
        for t in range(NT):
            n_ct = int(cnt[c, t])
            base = int(Q[t])
            sl = slice(lo + start, lo + start + n_ct)
            ks = np.arange(base, base + n_ct)
            sr[ks] = src_row[sl]
            reb = 128 * np.asarray([w1[k // 128] for k in ks], np.int64)
            dloc = dsts[sl] - c * N_SH
            dv = (dloc - reb).astype(np.float32)
            assert (dv >= 0).all() and (dv < 256).all()
            doff[ks] = dv
            dd[ks] = dists[sl]
            et[ks] = etypes[sl]
            start += n_ct
        assert start == core_hi[c] - core_lo[c]
        x3 = np.stack([dd, dd * dd, np.ones_like(dd)], 0).astype(np.float32)
        Zc = np.zeros(N_PAD, dtype=np.int64)
        Zc[:N_SH] = Z[c * N_SH:(c + 1) * N_SH]
        core_in.append(dict(
            src_row=to_pf(sr, n_sub), dst_off=to_pf(doff, n_sub), x3=x3,
            et16=wrap16(et), z16=wrap16(Zc),
            et32=to_pf(et.astype(np.int32), n_sub),
            z32=np.ascontiguousarray(Zc.astype(np.int32).reshape(NT, P).T),
        ))

    w = {}
    centers = np.linspace(CUT_LO, CUT_HI, NC, dtype=np.float32)
    gap = np.float32(centers[1] - centers[0])
    A = np.stack([2.0 * centers / gap,
                  -np.ones(NC, np.float32) / gap,
                  -(centers ** 2) / gap], 0).astype(np.float32)
    w["A"] = np.concatenate([A, np.zeros((3, 384 - NC), np.float32)], 1)
    w["node_emb16"] = np.asarray(inputs["node_emb"]).astype(NP16)
    w["edge_emb16"] = np.asarray(inputs["edge_emb"]).astype(NP16)
    for i in range(N_CONV):
        w[f"wn1t_{i}"] = np.ascontiguousarray(np.asarray(inputs["Wn1"][i]).T.astype(NP16))
        w[f"wn2t_{i}"] = np.ascontiguousarray(np.asarray(inputs["Wn2"][i]).T.astype(NP16))
        import ml_dtypes
        f8 = ml_dtypes.float8_e4m3
        we1t = np.zeros((512, 512), np.float32)      # K-pad 428->512, M-pad 428->512
        we1t[:DE, :DE] = np.asarray(inputs["We1"][i]).T
        we2t = np.zeros((512, D), np.float32)
        we2t[:DE, :] = np.asarray(inputs["We2"][i]).T
        for m in range(4):
            mc = we1t[:, m * 128:(m + 1) * 128]
            w[f"wdr_{i}_{m}_0"] = np.ascontiguousarray(
                np.stack([mc[0:128], mc[128:256]], 1).astype(f8))    # [128,2,128]
            w[f"wdr_{i}_{m}_1"] = np.ascontiguousarray(
                np.stack([mc[256:384], mc[384:512]], 1).astype(f8))
        w[f"w2dr_{i}_0"] = np.ascontiguousarray(
            np.stack([we2t[0:128], we2t[128:256]], 1).astype(f8))
        w[f"w2dr_{i}_1"] = np.ascontiguousarray(
            np.stack([we2t[256:384], we2t[384:512]], 1).astype(f8))
        w[f"wct_{i}"] = np.ascontiguousarray(np.asarray(inputs["Wc"][i]).T.astype(NP16))
        w[f"bn1_{i}"] = np.asarray(inputs["bn1"][i]).reshape(D, 1).astype(np.float32)
        w[f"bn2_{i}"] = np.asarray(inputs["bn2"][i]).reshape(D, 1).astype(np.float32)
        w[f"be1_{i}"] = np.asarray(inputs["be1"][i]).reshape(DE, 1).astype(np.float32)
        w[f"be2_{i}"] = np.asarray(inputs["be2"][i]).reshape(D, 1).astype(np.float32)
        w[f"bc_{i}"] = np.ascontiguousarray(
            np.tile(np.asarray(inputs["bc"][i]).reshape(1, D), (1, 4))).astype(NP16)
    w["wr1t"] = np.ascontiguousarray(np.asarray(inputs["Wr1"]).T.astype(np.float32))
    w["wr2t"] = np.ascontiguousarray(np.asarray(inputs["Wr2"]).T.astype(np.float32))
    w["br1"] = np.asarray(inputs["br1"]).reshape(D, 1).astype(np.float32)
    w["br2"] = np.full((D, 1), np.asarray(inputs["br2"]).reshape(()),
                       dtype=np.float32)

    # ---- 2-hop hn gather: epochs of SEG slots, 4 src chunks ----
    n_ep = (e_slots + SEG - 1) // SEG
    ck_all = []
    for c in range(NCORES):
        sr_flat = core_in[c]["src_row"].T.reshape(-1)      # slot order
        ck_all.append((sr_flat.astype(np.int64) // 32768).astype(np.int64))
    cnt_ec = np.zeros((NCORES, n_ep, 4), np.int64)
    for c in range(NCORES):
        for e in range(n_ep):
            seg = ck_all[c][e * SEG:(e + 1) * SEG]
            cnt_ec[c, e] = np.bincount(seg, minlength=4)
    P_ec = ((cnt_ec.max(axis=0) + 127) // 128) * 128       # [n_ep, 4]
    segb = np.zeros((n_ep, 5), np.int64)
    segb[:, 1:] = np.cumsum(P_ec, axis=1)
    T_ep = segb[:, 4]
    h1b = np.zeros(n_ep * 4 + 1, np.int64)
    h1b[1:] = np.cumsum(P_ec.reshape(-1))
    h1tot = int(h1b[-1])
    for c in range(NCORES):
        sr_flat = core_in[c]["src_row"].T.reshape(-1)
        ck = ck_all[c]
        h1 = np.zeros(h1tot, np.int16)
        h2 = np.zeros(e_slots, np.int64)
        for e in range(n_ep):
            lo_, hi_ = e * SEG, min((e + 1) * SEG, e_slots)
            seg_sr = sr_flat[lo_:hi_]
            seg_ck = ck[lo_:hi_]
            for k in range(4):
                sel = np.nonzero(seg_ck == k)[0]
                base = int(h1b[e * 4 + k])
                h1[base:base + len(sel)] = (seg_sr[sel] - 32768 * k).astype(np.int16)
                h2[lo_ + sel] = segb[e, k] + np.arange(len(sel))
        core_in[c]["h1"] = wrap16(h1)
        core_in[c]["h2"] = wrap16(h2)
    meta = (tuple(int(x) for x in q),
            tuple(tuple(int(x) for x in row) for row in P_ec))
    return core_in, w, meta, graph_ids


def build_nc(meta, reps=1):
    q, P_ec_t = meta
    P_ec = np.asarray(P_ec_t, np.int64)
    n_ep = P_ec.shape[0]
    segb = np.zeros((n_ep, 5), np.int64)
    segb[:, 1:] = np.cumsum(P_ec, axis=1)
    T_MAX = int(segb[:, 4].max())
    h1b = np.zeros(n_ep * 4 + 1, np.int64)
    h1b[1:] = np.cumsum(P_ec.reshape(-1))
    h1tot = int(h1b[-1])
    Q, e_slots, n_sub, n512, tiles_of, w1, s_first, s_last = geometry(q)

    nc = bass.Bass(num_devices=NCORES, num_swdge_queues=2)

    t_in = {}

    def inp(name, shp, dt=F32):
        t_in[name] = nc.dram_tensor(name, shp, dt, kind="ExternalInput")
        return t_in[name]

    src_row = inp("src_row", [P, n_sub], I32)
    dst_off = inp("dst_off", [P, n_sub], F32)
    x3 = inp("x3", [3, e_slots], F32)
    et16 = inp("et16", [P, e_slots // 16], I16)
    z16 = inp("z16", [P, N_PAD // 16], I16)
    et32 = inp("et32", [P, n_sub], I32)
    z32 = inp("z32", [P, NT], I32)
    h1_t = inp("h1", [P, h1tot // 16], I16)
    h2_t = inp("h2", [P, e_slots // 16], I16)
    A_t = inp("A", [3, 384], F32)
    node_emb16 = inp("node_emb16", [20, D], F16)
    edge_emb16 = inp("edge_emb16", [400, D], F16)
    for i in range(N_CONV):
        for nm, shp in (("bn1", [D, 1]), ("bn2", [D, 1]), ("be1", [DE, 1]),
                        ("be2", [D, 1])):
            inp(f"{nm}_{i}", shp)
        for nm, shp in (("wn1t", [D, D]), ("wn2t", [D, D]),
                        ("wct", [D, D]), ("bc", [1, 4 * D])):
            inp(f"{nm}_{i}", shp, F16)
        for m in range(4):
            inp(f"wdr_{i}_{m}_0", [P, 2, P], F8)
            inp(f"wdr_{i}_{m}_1", [P, 2, P], F8)
        inp(f"w2dr_{i}_0", [P, 2, P], F8)
        inp(f"w2dr_{i}_1", [P, 2, P], F8)
    inp("wr1t", [D, D]); inp("wr2t", [D, 1]); inp("br1", [D, 1]); inp("br2", [D, 1])
    r_out = nc.dram_tensor("r_out", [N_PAD, 1], F32, kind="ExternalOutput")

    cc_in = [nc.dram_tensor(f"cc_in_{i}", [N_PAD, D], F16, kind="Internal")
             for i in range(N_CONV)]
    cc_out = [nc.dram_tensor(f"cc_out_{i}", [NCORES * N_PAD, D], F16,
                             kind="Internal", addr_space="Shared")
              for i in range(N_CONV)]

    n_eg = (n512 + EG_GRP - 1) // EG_GRP     # e_fm gather launches per layer

    with tile.TileContext(nc) as tc:
        with (
            tc.tile_pool(name="const", bufs=1) as cp,
            tc.tile_pool(name="sb", bufs=3) as sb,
            tc.tile_pool(name="x3p", bufs=2) as xp,
            tc.tile_pool(name="gat", bufs=6) as gp,
            tc.tile_pool(name="egp", bufs=2) as ep,
            tc.tile_pool(name="tbl", bufs=2) as tbp,
            tc.tile_pool(name="zr", bufs=2) as zp,
            tc.tile_pool(name="pv", bufs=1, space="PSUM") as pv,
            tc.tile_pool(name="pz1", bufs=2, space="PSUM") as pz1,
            tc.tile_pool(name="pz2", bufs=1, space="PSUM") as pz2,
            tc.tile_pool(name="ptp", bufs=1, space="PSUM") as ptp,
            tc.tile_pool(name="pd", bufs=3, space="PSUM") as pd,
        ):
            ident = cp.tile([P, P], F32)
            make_identity(nc, ident[:])
            ident16 = cp.tile([P, P], F16)
            nc.vector.tensor_copy(out=ident16[:], in_=ident[:])
            iota_i = cp.tile([P, P], I32)
            nc.gpsimd.iota(iota_i[:], pattern=[[1, P]], base=0, channel_multiplier=0)
            iota_lo = cp.tile([P, P], F32)
            nc.vector.tensor_copy(out=iota_lo[:], in_=iota_i[:])
            iota_hi = cp.tile([P, P], F32)
            nc.vector.tensor_scalar(out=iota_hi[:], in0=iota_lo[:],
                                    scalar1=128.0, scalar2=None, op0=ALU.add)
            ones_row = cp.tile([1, P], F16)
            nc.vector.memset(ones_row[:], 1.0)

            def load_const(name, shp, dt=F32):
                tl = cp.tile(shp, dt, tag=name)
                nc.sync.dma_start(out=tl[:], in_=t_in[name][:, :])
                return tl

            A_sb = load_const("A", [3, 384])
            wr1t_sb = load_const("wr1t", [D, D])
            wr2t_sb = load_const("wr2t", [D, 1])
            br1_sb = load_const("br1", [D, 1])
            br2_sb = load_const("br2", [D, 1])
            W = {}
            for i in range(N_CONV):
                for nm in ("bn1", "bn2", "be2"):
                    W[f"{nm}_{i}"] = load_const(f"{nm}_{i}", [D, 1])
                for nm, shp in (("wn1t", [D, D]), ("wn2t", [D, D]),
                                ("wct", [D, D]), ("bc", [1, 4 * D])):
                    W[f"{nm}_{i}"] = load_const(f"{nm}_{i}", shp, F16)
                for m in range(4):
                    for pp in range(2):
                        tl = cp.tile([P, 2, P], F8, tag=f"wdr_{i}_{m}_{pp}")
                        nc.sync.dma_start(out=tl[:], in_=t_in[f"wdr_{i}_{m}_{pp}"][:, :, :])
                        W[f"wdr_{i}_{m}_{pp}"] = tl
                for pp in range(2):
                    tl = cp.tile([P, 2, P], F8, tag=f"w2dr_{i}_{pp}")
                    nc.sync.dma_start(out=tl[:], in_=t_in[f"w2dr_{i}_{pp}"][:, :, :])
                    W[f"w2dr_{i}_{pp}"] = tl
                be1 = cp.tile([P, 4], F32, tag=f"be1_{i}")
                nc.vector.memset(be1[:], 0.0)
                for k, (k0, k1) in enumerate(KCH):
                    nc.sync.dma_start(out=be1[:k1 - k0, k:k + 1],
                                      in_=t_in[f"be1_{i}"][k0:k1, :])
                W[f"be1_{i}"] = be1

            h_fm = cp.tile([P, N_PAD], F32, tag="h_fm")

            _regs = {}

            def nreg(v):
                if v not in _regs:
                    _regs[v] = nc.gpsimd.to_reg(v)
                return _regs[v]

            doff_sb = cp.tile([P, n_sub], F32, tag="doff_sb")
            nc.sync.dma_start(out=doff_sb[:], in_=dst_off[:, :])
            if "nodg" in ABLATE:
                sri = cp.tile([P, n_sub], I32, tag="sri_sb")
                nc.sync.dma_start(out=sri[:], in_=src_row[:, :])
            eti = cp.tile([P, e_slots // 16], I16, tag="eti_sb")
            nc.sync.dma_start(out=eti[:], in_=et16[:, :])
            zi16 = cp.tile([P, N_PAD // 16], I16, tag="z16_sb")
            nc.sync.dma_start(out=zi16[:], in_=z16[:, :])
            NODG = "nodg" in ABLATE
            if not NODG:
                h1_sb = cp.tile([P, h1tot // 16], I16, tag="h1_sb")
                nc.sync.dma_start(out=h1_sb[:], in_=h1_t[:, :])
                h2_sb = cp.tile([P, e_slots // 16], I16, tag="h2_sb")
                nc.sync.dma_start(out=h2_sb[:], in_=h2_t[:, :])
            if NODG:
                eti32 = cp.tile([P, n_sub], I32, tag="eti32_sb")
                nc.sync.dma_start(out=eti32[:], in_=et32[:, :])
                zi32 = cp.tile([P, NT], I32, tag="z32_sb")
                nc.sync.dma_start(out=zi32[:], in_=z32[:, :])

            def gather_transpose16(pt_wide, dst_col, table, idx_col):
                g = gp.tile([P, D], F16, tag="gath16")
                if "nogather" in ABLATE:
                    nc.gpsimd.dma_start(out=g[:], in_=table[0:P, :])
                else:
                    nc.gpsimd.indirect_dma_start(
                        out=g[:], out_offset=None, in_=table[:, :],
                        in_offset=bass.IndirectOffsetOnAxis(ap=idx_col, axis=0))
                nc.tensor.transpose(out=pt_wide[:, dst_col:dst_col + P], in_=g[:],
                                    identity=ident16[:])

            for rep_ in range(reps):
                # ---------- prologue: h0 ----------
                if NODG:
                    for t in range(NT):
                        ptw0 = ptp.tile([P, 512], F16, space="PSUM", tag="tpw")
                        gather_transpose16(ptw0, 0, node_emb16, zi32[:, t:t + 1])
                        nc.vector.tensor_copy(out=h_fm[:, t * P:(t + 1) * P],
                                              in_=ptw0[:, :P])
                else:
                    for a0 in range(0, N_PAD, H0_GRP):
                        nn = min(H0_GRP, N_PAD - a0)
                        hg = ep.tile([P, 1, H0_GRP], F16, tag="h0g")
                        nc.gpsimd.dma_gather(
                            out_ap=hg[:, :, :nn], in_ap=node_emb16[:, :],
                            idxs_ap=zi16[:, a0 // 16:(a0 + nn) // 16],
                            num_idxs=nn, num_idxs_reg=nreg(nn), elem_size=D,
                            transpose=True, queue_num=1)
                        nc.vector.tensor_copy(out=h_fm[:, a0:a0 + nn], in_=hg[:, 0, :nn])

                # ---------- layers ----------
                for i in range(N_CONV):
                    # --- node MLP (fp16) on own shard ---
                    for j0 in range(0, N_PAD, 512):
                        wdt = min(512, N_PAD - j0)
                        h16 = sb.tile([P, 512], F16, tag="h16")
                        nc.vector.tensor_copy(out=h16[:, :wdt], in_=h_fm[:, j0:j0 + wdt])
                        ps1 = pz1.tile([P, 512], F32, space="PSUM", tag="pz1")
                        nc.tensor.matmul(out=ps1[:, :wdt], lhsT=W[f"wn1t_{i}"][:],
                                         rhs=h16[:, :wdt], start=True, stop=True)
                        zb = sb.tile([P, 512], F16, tag="nmlp_z")
                        nc.scalar.activation(out=zb[:, :wdt], in_=ps1[:, :wdt],
                                             func=AF.Relu, bias=W[f"bn1_{i}"][:, :1])
                        ps2 = pz2.tile([P, 512], F32, space="PSUM", tag="pz2")
                        nc.tensor.matmul(out=ps2[:, :wdt], lhsT=W[f"wn2t_{i}"][:],
                                         rhs=zb[:, :wdt], start=True, stop=True)
                        hnb = sb.tile([P, 512], F16, tag="nmlp_hn")
                        nc.scalar.activation(out=hnb[:, :wdt], in_=ps2[:, :wdt],
                                             func=AF.Identity, bias=W[f"bn2_{i}"][:, :1])
                        ptw = ptp.tile([P, 512], F16, space="PSUM", tag="tpw")
                        for a in range(wdt // P):
                            nc.tensor.transpose(out=ptw[:, a * P:(a + 1) * P],
                                                in_=hnb[:, a * P:(a + 1) * P],
                                                identity=ident16[:])
                        hnm = sb.tile([P, 512], F16, tag="hn_nm")
                        nc.vector.tensor_copy(out=hnm[:, :wdt], in_=ptw[:, :wdt])
                        for a in range(wdt // P):
                            nc.sync.dma_start(
                                out=cc_in[i][j0 + a * P:j0 + (a + 1) * P, :],
                                in_=hnm[:, a * P:(a + 1) * P])

                    if "nocc" not in ABLATE:
                        nc.gpsimd.collective_compute(
                            "AllGather", ALU.bypass,
                            replica_groups=[list(range(NCORES))],
                            ins=[cc_in[i][:, :]], outs=[cc_out[i][:, :]])

                    # --- fused z-chain + consume per 512-tile ---
                    x3q = None
                    egt = None
                    tbl_t = None
                    hb = None
                    pd_map = {}
                    for j in range(n512):
                        js = slice(j * 512, (j + 1) * 512)
                        if not NODG and j % 16 == 0:
                            e = j // 16
                            tbl_t = tbp.tile([P, T_MAX // 128, P], F16, tag="hntbl")
                            for k in range(4):
                                lo_r = 32768 * k
                                hi_r = min(32768 * (k + 1), NCORES * N_PAD)
                                pk = int(P_ec[e, k])
                                for off in range(0, pk, 2048):
                                    nn = min(2048, pk - off)
                                    ob = int(segb[e, k]) + off
                                    ib = int(h1b[e * 4 + k]) + off
                                    nc.gpsimd.dma_gather(
                                        out_ap=tbl_t[:, ob // 128:(ob + nn) // 128, :],
                                        in_ap=cc_out[i][lo_r:hi_r, :],
                                        idxs_ap=h1_sb[:, ib // 16:(ib + nn) // 16],
                                        num_idxs=nn, num_idxs_reg=nreg(nn),
                                        elem_size=D, transpose=False)
                        if not NODG and j % EG_GRP == 0:
                            qw2 = min(EG_GRP * 512, e_slots - j * 512)
                            hb = ep.tile([P, 1, EG_GRP * 512], F16, tag="hnf2")
                            nc.gpsimd.dma_gather(
                                out_ap=hb[:, :, :qw2], in_ap=tbl_t[:],
                                idxs_ap=h2_sb[:, j * 32:j * 32 + qw2 // 16],
                                num_idxs=qw2, num_idxs_reg=nreg(qw2),
                                elem_size=D, transpose=True, queue_num=1,
                                sbuf_tokens_per_rank=128,
                                sbuf_free_dim_per_rank=256)
                        if j % EG_GRP == 0:
                            qw = min(EG_GRP * 512, e_slots - j * 512)
                            x3q = xp.tile([3, EG_GRP * 512], F32, tag="x3q")
                            nc.sync.dma_start(out=x3q[:, :qw],
                                              in_=x3[:, j * 512:j * 512 + qw])
                            if not NODG:
                                egt = ep.tile([P, 1, EG_GRP * 512], F16, tag="eg")
                                nc.gpsimd.dma_gather(
                                    out_ap=egt[:, :, :qw], in_ap=edge_emb16[:, :],
                                    idxs_ap=eti[:, j * 32:j * 32 + qw // 16],
                                    num_idxs=qw, num_idxs_reg=nreg(qw), elem_size=D,
                                    transpose=True, queue_num=1)
                        x3t = x3q[:, (j % EG_GRP) * 512:(j % EG_GRP + 1) * 512]
                        ev0 = zp.tile([P, 2, 512], F8, tag="ev0")
                        ev1 = zp.tile([P, 2, 512], F8, tag="ev1")
                        if NODG:
                            ptw0 = ptp.tile([P, 512], F16, space="PSUM", tag="tpw")
                            for a in range(4):
                                gather_transpose16(ptw0, a * P, edge_emb16,
                                                   eti32[:, j * 4 + a:j * 4 + a + 1])
                            nc.vector.tensor_copy(out=ev0[:, 0, :], in_=ptw0[:])
                        else:
                            nc.vector.tensor_copy(
                                out=ev0[:, 0, :],
                                in_=egt[:, 0, (j % EG_GRP) * 512:(j % EG_GRP + 1) * 512])
                        # V chunks = exp(A.T @ x3) into fp8 planes
                        vdst = (ev0[:, 1, :], ev1[:, 0, :], ev1[:, 1, :])
                        for k in range(3):
                            pvt = pv.tile([P, 512], F32, space="PSUM", tag="pv")
                            nc.tensor.matmul(out=pvt[:],
                                             lhsT=A_sb[:, k * 128:(k + 1) * 128],
                                             rhs=x3t, start=True, stop=True)
                            nc.scalar.activation(out=vdst[k], in_=pvt[:], func=AF.Exp)
                        # z1: 4 M-chunks x 2 DoubleRow matmuls (fp8)
                        z1pa = zp.tile([P, 2, 512], F8, tag="z1p0")
                        z1pb = zp.tile([P, 2, 512], F8, tag="z1p1")
                        z1p = [z1pa, z1pb]
                        for m in range(4):
                            pz = pz1.tile([P, 512], F32, space="PSUM", tag="pz1")
                            nc.tensor.matmul(out=pz[:], lhsT=W[f"wdr_{i}_{m}_0"][:],
                                             rhs=ev0[:], start=True, stop=False,
                                             perf_mode=mybir.MatmulPerfMode.DoubleRow)
                            nc.tensor.matmul(out=pz[:], lhsT=W[f"wdr_{i}_{m}_1"][:],
                                             rhs=ev1[:], start=False, stop=True,
                                             perf_mode=mybir.MatmulPerfMode.DoubleRow)
                            nc.vector.tensor_scalar(
                                out=z1p[m // 2][:, m % 2, :], in0=pz[:],
                                scalar1=W[f"be1_{i}"][:, m:m + 1],
                                scalar2=0.0, op0=ALU.add, op1=ALU.max)
                        # z2 = We2 @ z1 + be2 -> eet  (2 DoubleRow matmuls)
                        pe = pz2.tile([P, 512], F32, space="PSUM", tag="pz2")
                        nc.tensor.matmul(out=pe[:], lhsT=W[f"w2dr_{i}_0"][:],
                                         rhs=z1p[0][:], start=True, stop=False,
                                         perf_mode=mybir.MatmulPerfMode.DoubleRow)
                        nc.tensor.matmul(out=pe[:], lhsT=W[f"w2dr_{i}_1"][:],
                                         rhs=z1p[1][:], start=False, stop=True,
                                         perf_mode=mybir.MatmulPerfMode.DoubleRow)
                        eet = sb.tile([P, 512], F16, tag="eet")
                        nc.scalar.activation(out=eet[:], in_=pe[:], func=AF.Identity,
                                             bias=W[f"be2_{i}"][:, :1])

                        # consume: gather hn, product, message, scatter
                        prod = sb.tile([P, 512], F16, tag="prod")
                        if NODG:
                            ptw = ptp.tile([P, 512], F16, space="PSUM", tag="tpw")
                            for a in range(4):
                                s = j * 4 + a
                                gather_transpose16(ptw, a * P, cc_out[i], sri[:, s:s + 1])
                            hnf = sb.tile([P, 512], F16, tag="hnf")
                            nc.vector.tensor_copy(out=hnf[:], in_=ptw[:])
                            nc.vector.tensor_mul(out=prod[:], in0=eet[:], in1=hnf[:])
                        else:
                            nc.vector.tensor_mul(
                                out=prod[:], in0=eet[:],
                                in1=hb[:, 0, (j % EG_GRP) * 512:(j % EG_GRP + 1) * 512])
                        pmw = pz1.tile([P, 512], F32, space="PSUM", tag="pz1")
                        nc.tensor.matmul(out=pmw[:], lhsT=ones_row[:],
                                         rhs=W[f"bc_{i}"][:, :], start=True, stop=False)
                        for a in range(4):
                            nc.tensor.matmul(out=pmw[:, a * P:(a + 1) * P],
                                             lhsT=prod[:, a * P:(a + 1) * P],
                                             rhs=W[f"wct_{i}"][:], start=False,
                                             stop=True, skip_group_check=True)
                        msb = sb.tile([P, 512], F16, tag="msb")
                        nc.scalar.activation(out=msb[:], in_=pmw[:], func=AF.Tanh)

                        for a in range(4):
                            s = j * 4 + a
                            for t in tiles_of[s]:
                                wo = t - w1[s]          # 0 or 1
                                S = sb.tile([P, P], F16, tag="S")
                                nc.vector.tensor_tensor(
                                    out=S[:],
                                    in0=doff_sb[:, s:s + 1].to_broadcast([P, P]),
                                    in1=(iota_lo if wo == 0 else iota_hi)[:],
                                    op=ALU.is_equal)
                                if t not in pd_map:
                                    pd_map[t] = pd.tile([P, P], F32, space="PSUM",
                                                        tag="pd", name=f"pd_{i}_{t}")
                                nc.tensor.matmul(out=pd_map[t][:], lhsT=S[:],
                                                 rhs=msb[:, a * P:(a + 1) * P],
                                                 start=(s == s_first[t]),
                                                 stop=(s == s_last[t]))
                                if s == s_last[t]:
                                    dsb = sb.tile([P, P], F32, tag="dsb")
                                    nc.vector.tensor_copy(out=dsb[:], in_=pd_map[t][:])
                                    pt = pd.tile([P, P], F32, space="PSUM", tag="pd",
                                                 name=f"tp_{i}_{t}")
                                    nc.tensor.transpose(out=pt[:], in_=dsb[:],
                                                        identity=ident[:])
                                    nc.vector.tensor_add(
                                        out=h_fm[:, t * P:(t + 1) * P],
                                        in0=h_fm[:, t * P:(t + 1) * P], in1=pt[:])
                                    del pd_map[t]

                # ---------- readout ----------
                for j0 in range(0, N_PAD, 512):
                    wdt = min(512, N_PAD - j0)
                    ps1 = pz1.tile([P, 512], F32, space="PSUM", tag="pz1")
                    nc.tensor.matmul(out=ps1[:, :wdt], lhsT=wr1t_sb[:],
                                     rhs=h_fm[:, j0:j0 + wdt], start=True, stop=True)
                    qb = sb.tile([P, 512], F32, tag="qb")
                    nc.scalar.activation(out=qb[:, :wdt], in_=ps1[:, :wdt],
                                         func=AF.Relu, bias=br1_sb[:, :1])
                    for a in range(wdt // P):
                        prt = pz2.tile([P, 512], F32, space="PSUM", tag="pz2")
                        nc.tensor.matmul(out=prt[:, :1], lhsT=qb[:, a * P:(a + 1) * P],
                                         rhs=wr2t_sb[:], start=True, stop=True)
                        rsb = sb.tile([P, 1], F32, tag="rsb")
                        nc.scalar.activation(out=rsb[:], in_=prt[:, :1],
                                             func=AF.Identity, bias=br2_sb[:, :1])
                        nc.sync.dma_start(out=r_out[j0 + a * P:j0 + (a + 1) * P, :],
                                          in_=rsb[:])
    return nc


_CACHE = {}


def _get_runner(meta, reps=1):
    key = (meta, reps, ABLATE)
    if key not in _CACHE:
        nc = build_nc(meta, reps=reps)
        finalize_v2(nc)
        _CACHE[key] = nc
    return _CACHE[key]


def kernel(**inputs):
    core_in, w, meta, graph_ids = host_prep(inputs)
    nc = _get_runner(meta)
    in_maps = []
    for c in range(NCORES):
        m = dict(core_in[c])
        m.update(w)
        in_maps.append(m)
    res = run_bass_kernel_spmd(nc, in_maps, core_ids=list(range(NCORES)))
    r = np.concatenate([res.results[c]["r_out"][:N_SH, 0] for c in range(NCORES)])
    out = np.bincount(graph_ids, weights=r.astype(np.float64), minlength=G)[:G]
    return out.astype(np.float32)
